# revision 5
# baseline (speedup 1.0000x reference)
"""MiniMoERouter Trainium2 kernel: top-1 MoE, expert-parallel across 8 cores.

Math identity exploited: the reference runs every expert on masked (zeroed)
inputs, so for a token routed to expert e the other experts contribute only
their constant bias path c_i = relu(b1_i) @ W2_i (+ b2_i).  Hence

    out[t] = relu(x[t] @ W1_e + b1_e) @ W2_e + corr_e,
    corr_e = sum_{i != e} relu(b1_i) @ W2_i + sum_i b2_i

which is 1/8th the dense FLOPs (the headroom).  Core e runs expert e's FFN
over all tokens routed to it (padded to a fixed capacity), in f32 storage
with float32r TensorEngine matmuls (1 cycle/row at free-dim >= 256).
"""

import numpy as np
import concourse.bass as bass
import concourse.mybir as mybir
from concourse import bacc, tile
from concourse.bass_utils import run_bass_kernel_spmd

F32 = mybir.dt.float32
F32R = mybir.dt.float32r

E, D, H = 8, 1024, 4096
NTOK = 16384
CG = 2560            # per-expert token capacity (2 halves of 1280)
HALF = CG // 2       # 1280 = 10 tiles of 128 = 5 blocks of 256
NB = 256             # FFN1 moving free dim (>=256 keeps float32r at 1 cyc/row)
HBS = 512            # H streamed in blocks of 512 (4 chunks of 128)

import os
_USE_F32 = bool(int(os.environ.get("MOE_USE_F32", "0")))
_CACHED = {}


def _build_nc():
    nc = bacc.Bacc(None, target_bir_lowering=False, debug=True)
    MDT = F32 if _USE_F32 else F32R
    xgT = nc.dram_tensor("xgT", [D, CG], MDT, kind="ExternalInput")
    w1 = nc.dram_tensor("w1", [D, H], MDT, kind="ExternalInput")
    b1r = nc.dram_tensor("b1r", [128, H // 128], F32, kind="ExternalInput")
    w2 = nc.dram_tensor("w2", [H, D], MDT, kind="ExternalInput")
    corr = nc.dram_tensor("corr", [128, D], F32, kind="ExternalInput")
    y = nc.dram_tensor("y", [CG, D], F32, kind="ExternalOutput")

    n_hb = H // HBS              # 8 H-blocks
    n_hc = HBS // 128            # 4 128-chunks per H-block
    n_nb = HALF // NB            # 5 FFN1 token blocks per half
    n_tt = HALF // 128           # 10 FFN2 token tiles per half

    with tile.TileContext(nc) as tc:
        with (
            tc.tile_pool(name="xg", bufs=1) as xg_p,
            tc.tile_pool(name="wt", bufs=2) as w_p,
            tc.tile_pool(name="ht", bufs=2) as h_p,
            tc.tile_pool(name="yacc", bufs=1) as y_p,
            tc.tile_pool(name="cst", bufs=1) as c_p,
            tc.tile_pool(name="ph", bufs=3, space="PSUM") as ph_p,
            tc.tile_pool(name="py", bufs=4, space="PSUM") as py_p,
        ):
            b1_sb = c_p.tile([128, H // 128], F32, name="b1_sb")
            nc.sync.dma_start(out=b1_sb[:], in_=b1r[:])
            corr_sb = c_p.tile([128, D], F32, name="corr_sb")
            nc.sync.dma_start(out=corr_sb[:], in_=corr[:])

            for th in range(2):
                t0 = th * HALF
                # this half's gathered tokens, transposed: 8 D-chunks x [128, HALF]
                xg_t = [
                    xg_p.tile([128, HALF], MDT, tag=f"xg{dc}", name=f"xg{dc}_{th}")
                    for dc in range(8)
                ]
                for dc in range(8):
                    nc.sync.dma_start(
                        out=xg_t[dc][:],
                        in_=xgT[dc * 128 : (dc + 1) * 128, t0 : t0 + HALF],
                    )
                y_t = [
                    y_p.tile([128, D], F32, tag=f"y{tt}", name=f"y{tt}_{th}")
                    for tt in range(n_tt)
                ]
                for hb in range(n_hb):
                    w1_t = [
                        w_p.tile([128, HBS], MDT, tag=f"w1{dc}", name=f"w1{dc}_{th}_{hb}")
                        for dc in range(8)
                    ]
                    for dc in range(8):
                        nc.sync.dma_start(
                            out=w1_t[dc][:],
                            in_=w1[dc * 128 : (dc + 1) * 128, hb * HBS : (hb + 1) * HBS],
                        )
                    w2_t = [
                        w_p.tile([128, D], MDT, tag=f"w2{hc}", name=f"w2{hc}_{th}_{hb}")
                        for hc in range(n_hc)
                    ]
                    for hc in range(n_hc):
                        r0 = hb * HBS + hc * 128
                        nc.sync.dma_start(out=w2_t[hc][:], in_=w2[r0 : r0 + 128, :])

                    # FFN1: hT[hc] = relu(W1_blk.T @ xg + b1)  -> [128 H, HALF tok]
                    h_t = [
                        h_p.tile([128, HALF], MDT, tag=f"h{hc}", name=f"h{hc}_{th}_{hb}")
                        for hc in range(n_hc)
                    ]
                    for hc in range(n_hc):
                        for nb in range(n_nb):
                            ph = ph_p.tile([128, NB], F32, tag="ph", name=f"ph_{th}_{hb}_{hc}_{nb}")
                            for dc in range(8):
                                nc.tensor.matmul(
                                    out=ph[:],
                                    lhsT=w1_t[dc][:, hc * 128 : (hc + 1) * 128],
                                    rhs=xg_t[dc][:, nb * NB : (nb + 1) * NB],
                                    start=(dc == 0),
                                    stop=(dc == 7),
                                )
                            nc.scalar.activation(
                                out=h_t[hc][:, nb * NB : (nb + 1) * NB],
                                in_=ph[:],
                                func=mybir.ActivationFunctionType.Relu,
                                bias=b1_sb[:, hb * n_hc + hc : hb * n_hc + hc + 1],
                            )

                    # FFN2 partial: y_t[tt] (+)= hT_blk.T @ W2_blk
                    for tt in range(n_tt):
                        for dn in range(2):
                            py = py_p.tile([128, 512], F32, tag="py", name=f"py_{th}_{hb}_{tt}_{dn}")
                            for hc in range(n_hc):
                                nc.tensor.matmul(
                                    out=py[:],
                                    lhsT=h_t[hc][:, tt * 128 : (tt + 1) * 128],
                                    rhs=w2_t[hc][:, dn * 512 : (dn + 1) * 512],
                                    start=(hc == 0),
                                    stop=(hc == n_hc - 1),
                                )
                            ys = y_t[tt][:, dn * 512 : (dn + 1) * 512]
                            if hb == 0:
                                nc.vector.tensor_add(
                                    out=ys, in0=py[:], in1=corr_sb[:, dn * 512 : (dn + 1) * 512]
                                )
                            else:
                                nc.vector.tensor_add(out=ys, in0=py[:], in1=ys)

                for tt in range(n_tt):
                    r0 = t0 + tt * 128
                    nc.sync.dma_start(out=y[r0 : r0 + 128, :], in_=y_t[tt][:])
    nc.compile()
    nc.finalize()
    return nc


def kernel(x, W1, b1, W2, b2, Wr, br):
    x = np.ascontiguousarray(np.asarray(x, dtype=np.float32))
    W1 = np.asarray(W1, dtype=np.float32)
    b1 = np.asarray(b1, dtype=np.float32)
    W2 = np.asarray(W2, dtype=np.float32)
    b2 = np.asarray(b2, dtype=np.float32)
    Wr = np.asarray(Wr, dtype=np.float32)
    br = np.asarray(br, dtype=np.float32)

    B, S, Dd = x.shape
    x2 = x.reshape(-1, Dd)

    # Router on host in f64 (0.1% of FLOPs): argmax is exact, matches f32 ref
    # (min top-2 gap ~2e-5 >> f32 rounding).
    logits = x2.astype(np.float64) @ Wr.astype(np.float64) + br.astype(np.float64)
    idx = np.argmax(logits, axis=1)

    # Expert bias paths (constant per expert): corr_e
    rb1 = np.maximum(b1.astype(np.float64), 0.0)          # [E, H]
    c = np.einsum("eh,ehd->ed", rb1, W2.astype(np.float64))  # [E, D]
    corr = (c.sum(0)[None, :] - c) + b2.astype(np.float64).sum(0)[None, :]  # [E, D]

    ids_list, in_maps = [], []
    for e in range(E):
        ids = np.nonzero(idx == e)[0].astype(np.int64)
        assert len(ids) <= CG, f"capacity overflow: expert {e} has {len(ids)} tokens"
        ids_list.append(ids)
        xg = np.zeros((CG, Dd), dtype=np.float32)
        xg[: len(ids)] = x2[ids]
        in_maps.append(
            {
                "xgT": np.ascontiguousarray(xg.T),
                "w1": np.ascontiguousarray(W1[e]),
                "b1r": np.ascontiguousarray(b1[e].reshape(H // 128, 128).T),
                "w2": np.ascontiguousarray(W2[e]),
                "corr": np.ascontiguousarray(
                    np.broadcast_to(corr[e].astype(np.float32), (128, Dd))
                ),
            }
        )

    if "nc" not in _CACHED:
        _CACHED["nc"] = _build_nc()
    _CACHED["in_maps"] = in_maps
    res = run_bass_kernel_spmd(_CACHED["nc"], in_maps, core_ids=list(range(E)))

    out = np.zeros((NTOK, Dd), dtype=np.float32)
    for e in range(E):
        ids = ids_list[e]
        out[ids] = res.results[e]["y"][: len(ids)]
    return out.reshape(B, S, Dd)
